# revision 13
# baseline (speedup 1.0000x reference)
"""DCGRU cell on 8 Trainium2 NeuronCores.

Strategy (data-parallel over batch B=64 -> 8 per core):
  - Sparse supports densified on host into S^T [2048, 2048], pre-scaled
    x512 and quantized to fp8e4 (e4m3); both supports stay SBUF-resident
    (loaded once, column-chunked DMAs so the first spmm starts early).
    All diffusion spmms run as fp8 DoubleRow matmuls (K=256 per
    instruction, fp32 PSUM accumulate).
  - Input features never touch the device diffusion: their projection
    contribution (identical for gate and candidate) is computed on the
    host in f32 and DMA-preloaded into each projection PSUM tile
    (matmuls then accumulate with start=False).
  - Activations live node-major [n, (b,u)], 512 cols: x0 in bf16 (+ a
    host-made fp8 x16 copy); x1 is stored ONLY as fp8(16*x1); the
    Chebyshev term is kept as 8192*x2' = S8@x1q - 4096*x0 in bf16.
  - Projection W is host-scaled by 256 (activation applies scale=1/256),
    which puts the fp8 copy of the m1/m3 rows (16*W) in prime e4m3
    range: m1+m3 contract as ONE fp8 DoubleRow matmul against the
    fp8 xs^T tiles (values x16, scales cancel by construction).
    Per (b, n-chunk): preload-DMA + 3 bf16 K=64 matmuls + 1 fp8
    DoubleRow K=128.
  - Sink stages are software-pipelined one block behind the matmuls so
    the PE never stalls on Scalar/Vector PSUM evacuations.
  - Gate: r^T is multiplied into x0 in place, u^T round-trips DRAM in
    bf16 (prefetched back).  The candidate w_stage fuses the GRU
    combine: per (b, n-half) it transposes c, streams the bf16 state
    (same DRAM tensor as x0), and writes the output directly.
"""

import numpy as np

import concourse.bass as bass
from concourse import bacc
import concourse.mybir as mybir
import concourse.tile as tile
from concourse.bass_utils import run_bass_kernel_spmd
from concourse.masks import make_identity

N = 2048            # nodes
B = 64              # global batch
BL = 8              # batch per core
NCORES = 8
D_IN = 2
U = 64              # hidden units
M = 5               # 1 + 2 supports * K
F = D_IN + U        # 66
NB = N // 128       # 16 node blocks
SC = BL * U         # 512 state cols in natural layout

F32 = mybir.dt.float32
BF16 = mybir.dt.bfloat16
FP8 = mybir.dt.float8e4

S_SCALE = 512.0     # S^T fp8 pre-scale (host)
X_SCALE = 16.0      # x fp8 pre-scale
PROD = S_SCALE * X_SCALE   # psum scale of S8 @ xq
W_SCALE = 256.0     # projection W pre-scale (activation descales)


def _build_nc():
    nc = bacc.Bacc(None, target_bir_lowering=False)

    x0d = nc.declare_dram_parameter("x0", [N, SC], BF16, isOutput=False)
    x0qd = nc.declare_dram_parameter("x0q", [N, SC], FP8, isOutput=False)
    sad = nc.declare_dram_parameter("sat8", [N, N], FP8, isOutput=False)
    sbd = nc.declare_dram_parameter("sbt8", [N, N], FP8, isOutput=False)
    wgsd = nc.declare_dram_parameter("wgs", [128, 3 * 128], BF16, isOutput=False)
    wg8d = nc.declare_dram_parameter("wg8", [128, 2 * 128], FP8, isOutput=False)
    wcsd = nc.declare_dram_parameter("wcs", [128, 3 * U], BF16, isOutput=False)
    wc8d = nc.declare_dram_parameter("wc8", [128, 2 * U], FP8, isOutput=False)
    bgd = nc.declare_dram_parameter("bg", [128, 1], F32, isOutput=False)
    bcd = nc.declare_dram_parameter("bc", [U, 1], F32, isOutput=False)
    pgid = nc.declare_dram_parameter("pgi", [BL * 128, N], F32, isOutput=False)
    pcid = nc.declare_dram_parameter("pci", [BL * U, N], F32, isOutput=False)
    outd = nc.declare_dram_parameter("out", [N, SC], F32, isOutput=True)
    ubufd = nc.dram_tensor("ubuf", [128, NB * SC], BF16)

    with tile.TileContext(nc) as tc:
        _emit(nc, tc, x0d, x0qd, sad, sbd, wgsd, wg8d, wcsd, wc8d,
              bgd, bcd, pgid, pcid, outd, ubufd)
    nc.compile()
    return nc


def _emit(nc, tc, x0d, x0qd, sad, sbd, wgsd, wg8d, wcsd, wc8d, bgd, bcd,
          pgid, pcid, outd, ubufd):
    from contextlib import ExitStack
    ctx = ExitStack()
    with ctx:
        consts = ctx.enter_context(tc.tile_pool(name="consts", bufs=1))
        nat = ctx.enter_context(tc.tile_pool(name="nat", bufs=1))
        x2p = ctx.enter_context(tc.tile_pool(name="x2p", bufs=2))
        small = ctx.enter_context(tc.tile_pool(name="small", bufs=2))
        cbp = ctx.enter_context(tc.tile_pool(name="cbp", bufs=4))
        stp = ctx.enter_context(tc.tile_pool(name="stp", bufs=2))
        utp = ctx.enter_context(tc.tile_pool(name="utp", bufs=2))
        ubp = ctx.enter_context(tc.tile_pool(name="ubp", bufs=2))
        tfp = ctx.enter_context(tc.tile_pool(name="tfp", bufs=2))
        pinp = ctx.enter_context(tc.tile_pool(name="pinp", bufs=4))
        psum = ctx.enter_context(tc.tile_pool(name="psum", bufs=8, space="PSUM"))

        identb = consts.tile([128, 128], BF16)
        make_identity(nc, identb[:])
        ident8 = consts.tile([128, 128], FP8)
        nc.vector.tensor_copy(ident8[:], identb[:])

        wgs = consts.tile([128, 3 * 128], BF16)
        wg8 = consts.tile([128, 2 * 128], FP8)
        wcs = consts.tile([128, 3 * U], BF16)
        wc8 = consts.tile([128, 2 * U], FP8)
        bg = consts.tile([128, 1], F32)
        bc = consts.tile([U, 1], F32)

        # natural-layout activations: block i at cols i*SC
        x0sb = nat.tile([128, NB * SC], BF16, tag="x0")
        x0q = nat.tile([128, NB * SC], FP8, tag="x0q")
        x1q = nat.tile([128, NB * SC], FP8, tag="x1q")
        sa8 = nat.tile([128, NB * N], FP8, tag="sa8")
        sb8 = nat.tile([128, NB * N], FP8, tag="sb8")
        # xs^T: bf16 for m in {0, 2, 4} (idx 0,1,2), fp8 for {1, 3} (idx 0,1)
        xsts = nat.tile([128, 3 * 4 * N], BF16, tag="xsts")
        xsts8 = nat.tile([128, 2 * 4 * N], FP8, tag="xsts8")

        # startup DMAs: x0 first (feeds m0 transposes), S in column chunks
        x0dv = x0d.rearrange("(j p) c -> p j c", p=128)
        x0sb3 = x0sb[:].rearrange("p (j c) -> p j c", j=NB)
        for g in range(4):
            nc.sync.dma_start(x0sb3[:, 4 * g:4 * g + 4, :],
                              x0dv[:, 4 * g:4 * g + 4, :])
        nc.sync.dma_start(
            x0q[:].rearrange("p (j c) -> p j c", j=NB),
            x0qd.rearrange("(j p) c -> p j c", p=128))
        sa3 = sa8[:].rearrange("p (j c) -> p j c", j=NB)
        sadv = sad.rearrange("(j p) c -> p j c", p=128)
        sb3 = sb8[:].rearrange("p (j c) -> p j c", j=NB)
        sbdv = sbd.rearrange("(j p) c -> p j c", p=128)
        for g in range(4):
            nc.sync.dma_start(sa3[:, :, 512 * g:512 * g + 512],
                              sadv[:, :, 512 * g:512 * g + 512])
        for g in range(4):
            nc.sync.dma_start(sb3[:, :, 512 * g:512 * g + 512],
                              sbdv[:, :, 512 * g:512 * g + 512])
        for dst, src in ((wgs, wgsd), (wg8, wg8d), (wcs, wcsd), (wc8, wc8d),
                         (bg, bgd), (bc, bcd)):
            nc.sync.dma_start(dst[:], src[:])

        stfv = x0d.rearrange("(i p) c -> p i c", p=128)
        outv = outd.rearrange("(i p) c -> p i c", p=128)
        x1q3 = x1q[:].rearrange("p (j c) -> p j c", j=NB)
        xst83 = xsts8[:].rearrange("p (mi j n) -> p mi j n", mi=2, n=N)
        wg83 = wg8[:].rearrange("p (two o) -> p two o", two=2)
        wc83 = wc8[:].rearrange("p (two o) -> p two o", two=2)

        def xst_s(mi, j):
            return xsts[:, (mi * 4 + j) * N:(mi * 4 + j + 1) * N]

        def xst8_s(mi, j):
            return xsts8[:, (mi * 4 + j) * N:(mi * 4 + j + 1) * N]

        def xst_transposes(sel, i, src_ap):
            """4 PE transposes of natural block i into xs^T."""
            is8, mi = sel
            idt = ident8 if is8 else identb
            dst = xst8_s if is8 else xst_s
            for j in range(4):
                if is8:
                    # fp8 transpose requires psum element step of 2
                    pt = psum.tile([128, 256], FP8, tag="ps")
                    pv = pt[:].rearrange("p (c two) -> p c two", two=2)[:, :, 0]
                else:
                    pt = psum.tile([128, 128], BF16, tag="ps")
                    pv = pt[:]
                nc.tensor.transpose(
                    pv, src_ap[:, j * 128:(j + 1) * 128], idt[:])
                nc.vector.tensor_copy(
                    dst(mi, j)[:, i * 128:(i + 1) * 128], pv)

        def spmm(s8, xq, sink, after_flushed=None):
            """Y = S8 @ Xq via fp8 DoubleRow (K=256/instr), fp32 PSUM.
            sink(i, pt) -> deferred PE work, pipelined one block behind.
            after_flushed(k) fires once block k's deferred work is
            emitted (used to interleave projection chunks)."""
            s3 = s8[:].rearrange("p (j c) -> p j c", j=NB)
            xq3 = xq[:].rearrange("p (j c) -> p j c", j=NB)
            pending = None
            for i in range(NB):
                pt = psum.tile([128, 512], F32, tag="ps", name=f"pmm{i}")
                for jj in range(NB // 2):
                    nc.tensor.matmul(
                        pt[:],
                        s3[:, 2 * jj:2 * jj + 2, i * 128:(i + 1) * 128],
                        xq3[:, 2 * jj:2 * jj + 2, :],
                        start=(jj == 0), stop=(jj == NB // 2 - 1),
                        perf_mode=mybir.MatmulPerfMode.DoubleRow)
                if pending is not None:
                    pending()
                    if after_flushed is not None:
                        after_flushed(i - 1)
                pending = sink(i, pt)
            pending()
            if after_flushed is not None:
                after_flushed(NB - 1)

        def dconv(after_flushed=None, skip_m0=False):
            if not skip_m0:
                for i in range(NB):
                    xst_transposes((False, 0), i,
                                   x0sb[:, i * SC:(i + 1) * SC])
            for s, s8 in ((0, sa8), (1, sb8)):

                def x1_sink(i, pt, s=s):
                    xb = x1q3[:, i]
                    nc.scalar.mul(xb, pt[:], 1.0 / S_SCALE)

                    def deferred():
                        xst_transposes((True, s), i,
                                       x1q[:, i * SC:(i + 1) * SC])
                    return deferred

                spmm(s8, x0q, x1_sink)

                def x2_sink(i, pt, s=s):
                    blk = x2p.tile([128, SC], BF16, tag="x2")
                    nc.vector.scalar_tensor_tensor(
                        blk[:], x0sb[:, i * SC:(i + 1) * SC],
                        -(PROD / 2.0), pt[:],
                        mybir.AluOpType.mult, mybir.AluOpType.add)

                    def deferred():
                        xst_transposes((False, 1 + s), i, blk)
                    return deferred

                spmm(s8, x1q, x2_sink,
                     after_flushed if s == 1 else None)

        def proj_mms(b, c, gate):
            """Projection psum for (b, n-chunk c): host-input preload +
            3 bf16 K=64 matmuls + 1 fp8 DoubleRow (m1+m3)."""
            ws, w83, pind, O = ((wgs, wg83, pgid, 128) if gate
                                else (wcs, wc83, pcid, U))
            pin = pinp.tile([O, 512], F32, tag="pin")
            nc.sync.dma_start(
                pin[:], pind[b * O:(b + 1) * O, c * 512:(c + 1) * 512])
            pt = psum.tile([O, 512], F32, tag="ps", name="po")
            bp = (b % 2) * U
            for g in range(3):
                rs = xst_s(g, b // 2)[bp:bp + U, c * 512:(c + 1) * 512]
                nc.tensor.matmul(pt[:], ws[bp:bp + U, g * O:(g + 1) * O],
                                 rs, start=(g == 0), stop=False)
            for t in range(2):
                r8 = xst83[bp:bp + U, t, b // 2, c * 512:(c + 1) * 512]
                nc.tensor.matmul(pt[:], w83[bp:bp + U, t, :O], r8,
                                 start=False, stop=(t == 1))
            # add the host-computed input-feature contribution
            nc.vector.scalar_tensor_tensor(
                pt[:], pin[:], 1.0, pt[:],
                mybir.AluOpType.mult, mybir.AluOpType.add)
            return pt

        SIG = mybir.ActivationFunctionType.Sigmoid
        ubv = ubufd.rearrange("p (i b u) -> p i b u", b=BL, u=U)
        wsg_pending = [None]

        def gate_chunk(c):
            """Projection+sigmoid for n-chunk c (all b); r into x0sb, u to
            DRAM; then candX quantize + cand m0 transposes for the chunk.
            Emitted from inside the gate Sb-x2 spmm (after_flushed)."""
            for b in range(BL):
                pt = proj_mms(b, c, True)
                rub = small.tile([128, 512], BF16, tag="rub")
                nc.scalar.activation(rub[0:U, :], pt[:U, :], SIG,
                                     bias=bg[:U, :], scale=1.0 / W_SCALE)
                nc.scalar.activation(rub[U:128, :], pt[U:128, :], SIG,
                                     bias=bg[U:128, :], scale=1.0 / W_SCALE)

                def deferred(b=b, c=c, rub=rub):
                    rpt = psum.tile([128, 512], BF16, tag="ps")
                    for j in range(4):
                        nc.tensor.transpose(
                            rpt[:, j * 128:(j + 1) * 128],
                            rub[:, j * 128:(j + 1) * 128], identb[:])
                    rp4 = rpt[:].rearrange("p (j k u) -> p j k u", k=2, u=U)
                    xv = x0sb[:].rearrange("p (i c) -> p i c", c=SC)[
                        :, 4 * c:4 * c + 4, b * U:(b + 1) * U]
                    nc.vector.tensor_mul(xv, xv, rp4[:, :, 0, :])
                    ubb = ubp.tile([128, 256], BF16, tag="ubb")
                    nc.vector.tensor_copy(
                        ubb[:].rearrange("p (j u) -> p j u", u=U),
                        rp4[:, :, 1, :])
                    nc.sync.dma_start(
                        ubv[:, 4 * c:4 * c + 4, b, :],
                        ubb[:].rearrange("p (j u) -> p j u", u=U))
                if wsg_pending[0] is not None:
                    wsg_pending[0]()
                wsg_pending[0] = deferred
            # flush b=7 now: gate_chunk_post reads x0sb after its r-mult
            wsg_pending[0]()
            wsg_pending[0] = None

        def gate_chunk_post(c):
            """After chunk c's r-mults: quantize candX and emit the
            candidate dconv's m0 transposes for those blocks."""
            for i in range(4 * c, 4 * c + 4):
                nc.scalar.mul(x0q[:, i * SC:(i + 1) * SC],
                              x0sb[:, i * SC:(i + 1) * SC], X_SCALE)
                xst_transposes((False, 0), i, x0sb[:, i * SC:(i + 1) * SC])

        wsc_pending = [None]

        def cand_half(h):
            """Candidate projection + tanh + fused GRU combine for n-half
            h (blocks 8h..8h+8), emitted inside the cand Sb-x2 spmm."""
            for b in range(BL):
                stt = stp.tile([128, 512], BF16, tag="stt")
                nc.sync.dma_start(
                    stt[:].rearrange("p (i u) -> p i u", u=U),
                    stfv[:, 8 * h:8 * h + 8, b * U:(b + 1) * U])
                ut = utp.tile([128, 512], BF16, tag="ut")
                nc.sync.dma_start(
                    ut[:].rearrange("p (i u) -> p i u", u=U),
                    ubv[:, 8 * h:8 * h + 8, b, :])
                cbs = []
                for cc in range(2):
                    pt = proj_mms(b, 2 * h + cc, False)
                    cb = cbp.tile([U, 512], BF16, tag="cb")
                    nc.scalar.activation(
                        cb[:], pt[:, :],
                        mybir.ActivationFunctionType.Tanh, bias=bc[:],
                        scale=1.0 / W_SCALE)
                    cbs.append(cb)

                def deferred(b=b, h=h, cbs=cbs, stt=stt, ut=ut):
                    cpt = psum.tile([128, 512], BF16, tag="ps")
                    for cc in range(2):
                        for j in range(4):
                            nc.tensor.transpose(
                                cpt[:, cc * 256 + j * U:
                                    cc * 256 + (j + 1) * U],
                                cbs[cc][:, j * 128:(j + 1) * 128],
                                identb[:U, :U])
                    tf = tfp.tile([128, 512], F32, tag="tf")
                    # tf = (state - c) * u + c
                    nc.vector.tensor_sub(tf[:], stt[:], cpt[:])
                    nc.vector.tensor_mul(tf[:], tf[:], ut[:])
                    nc.vector.tensor_add(tf[:], tf[:], cpt[:])
                    nc.sync.dma_start(
                        outv[:, 8 * h:8 * h + 8, b * U:(b + 1) * U],
                        tf[:].rearrange("p (i u) -> p i u", u=U))
                if wsc_pending[0] is not None:
                    wsc_pending[0]()
                wsc_pending[0] = deferred
            if h == 1:
                wsc_pending[0]()
                wsc_pending[0] = None

        # ---- gate ----
        dconv()
        for c in range(4):
            gate_chunk(c)
            gate_chunk_post(c)
        # ---- candidate (x0q/m0 already prepared by gate_chunk_post) ----
        dconv(skip_m0=True)
        for h in range(2):
            cand_half(h)

_NC_CACHE = {}


def _get_nc():
    if "nc" not in _NC_CACHE:
        _NC_CACHE["nc"] = _build_nc()
    return _NC_CACHE["nc"]


def _host_prep(inputs, state, edges1, vals1, edges2, vals2, W_gate, b_gate,
               W_cand, b_cand):
    import ml_dtypes
    BF = ml_dtypes.bfloat16
    E4 = ml_dtypes.float8_e4m3
    inputs = np.asarray(inputs, np.float32)
    state = np.asarray(state, np.float32)
    Wg = np.asarray(W_gate, np.float32).reshape(F, M, 2 * U)
    Wc = np.asarray(W_cand, np.float32).reshape(F, M, U)

    def densify(edges, vals, transpose):
        S = np.zeros((N, N), np.float32)
        r, c = (1, 0) if transpose else (0, 1)
        np.add.at(S, (np.asarray(edges[r]).astype(np.int64),
                      np.asarray(edges[c]).astype(np.int64)),
                  np.asarray(vals, np.float32))
        return S

    SaT = densify(edges1, vals1, True)
    SbT = densify(edges2, vals2, True)
    SaT8 = (SaT * S_SCALE).astype(E4)
    SbT8 = (SbT * S_SCALE).astype(E4)

    def reorder(Wm):
        O = Wm.shape[2]
        # bf16 groups m0/m2/m4 (x W_SCALE; m2/m4 also 2/PROD for the
        # 8192*x2' storage scale), duplicated at partition bases 0/64
        Ws = np.empty((U, 3, O), np.float32)
        Ws[:, 0] = Wm[D_IN:, 0] * W_SCALE
        Ws[:, 1] = Wm[D_IN:, 2] * (W_SCALE * 2.0 / PROD)
        Ws[:, 2] = Wm[D_IN:, 4] * (W_SCALE * 2.0 / PROD)
        Ws = Ws.reshape(U, 3 * O)
        Ws2 = np.concatenate([Ws, Ws], 0)
        # fp8 pair m1/m3: 16*W against xs values 16*x1 -> 256 = W_SCALE
        W8 = np.empty((U, 2, O), np.float32)
        W8[:, 0] = Wm[D_IN:, 1] * (W_SCALE / X_SCALE)
        W8[:, 1] = Wm[D_IN:, 3] * (W_SCALE / X_SCALE)
        W8 = W8.reshape(U, 2 * O)
        W82 = np.concatenate([W8, W8], 0)
        return Ws2.astype(BF), W82.astype(E4)

    wgs, wg8 = reorder(Wg)
    wcs, wc8 = reorder(Wc)
    bgv = np.asarray(b_gate, np.float32).reshape(128, 1)
    bcv = np.asarray(b_cand, np.float32).reshape(U, 1)

    # host-side input-feature diffusion + projection (exact f32)
    Sa = densify(edges1, vals1, False)
    Sb = densify(edges2, vals2, False)
    Z = np.ascontiguousarray(
        inputs.reshape(B, N, D_IN).transpose(1, 0, 2).reshape(N, B * D_IN))
    z1a = Sa @ Z
    z2a = 2.0 * (Sa @ z1a) - Z
    z1b = Sb @ Z
    z2b = 2.0 * (Sb @ z1b) - Z
    xs_in = np.stack([Z, z1a, z2a, z1b, z2b], 0).reshape(M, N, B, D_IN)
    # pg[b, o, n] = sum_{m, fi} xs_in[m, n, b, fi] * W[fi, m, o]
    pg = np.einsum('mnbf,fmo->bon', xs_in, Wg[:D_IN] * W_SCALE,
                   optimize=True).astype(np.float32)
    pc = np.einsum('mnbf,fmo->bon', xs_in, Wc[:D_IN] * W_SCALE,
                   optimize=True).astype(np.float32)

    in_maps = []
    for cid in range(NCORES):
        bsl = slice(cid * BL, (cid + 1) * BL)
        st_c = state[bsl].reshape(BL, N, U)
        x0 = np.ascontiguousarray(st_c.transpose(1, 0, 2).reshape(N, SC))
        in_maps.append(dict(
            x0=x0.astype(BF),
            x0q=(x0 * X_SCALE).astype(E4),
            sat8=SaT8, sbt8=SbT8, wgs=wgs, wg8=wg8, wcs=wcs, wc8=wc8,
            bg=bgv, bc=bcv,
            pgi=np.ascontiguousarray(pg[bsl].reshape(BL * 128, N)),
            pci=np.ascontiguousarray(pc[bsl].reshape(BL * U, N)),
        ))
    return in_maps


def kernel(**inputs):
    nc = _get_nc()
    in_maps = _host_prep(**inputs)
    res = run_bass_kernel_spmd(nc, in_maps, list(range(NCORES)))
    outs = []
    for c in range(NCORES):
        o = np.asarray(res.results[c]["out"])          # [N, (b, u)]
        outs.append(o.reshape(N, BL, U).transpose(1, 0, 2).reshape(BL, N * U))
    return np.concatenate(outs, 0).astype(np.float32)
